# revision 88
# baseline (speedup 1.0000x reference)
"""Trainium2 Bass kernel for local windowed MHA (nn_LocalMHA), v2.

Computation (see reference): x (C=1024, T=16384) -> LayerNorm over C ->
QKV proj -> rotary (window-relative) -> per-head attention within windows
of 32 tokens -> out proj -> +x residual.

Sharding: T split across 8 cores (2048 tokens each); windows are local so
no communication is needed. Weights replicated.

v2 changes vs baseline (382us -> 301us in the TimelineSim cost model):
  - QKV and out-projection run in fp8e4 DoubleRow perf mode (2 k-tiles
    per matmul at 0.5 cycles/row, 4x cheaper than bf16) with one-term
    error-feedback weight compensation: W ~= W_hi8 + W_lo8 accumulated
    in the same PSUM group, which drops the weight-side quantization
    error into e4m3 subnormals (~0.1% rms).  Activations are scaled x8
    and weights x64 to stay in e4m3 normal range; the 512x psum scale
    is folded into the exp scale / ao eviction / residual stt, so the
    q/k/v evictions are plain copies.
  - Window masking is free: a rank-4 "onehot" matmul adds +BIGRAW to
    the in-window diagonal of every S block before exp, so off-window
    garbage underflows to 0 after the 1/Z normalize (no mask op).
  - Softmax: exp on ACT; per-group Z via 4x tensor_scalar with
    accum_out into a scratch tile; reciprocal; per-group 4x
    tensor_scalar normalize with the per-partition 1/Z; DVE 32x32
    stream transpose (exact for the block-diagonal P).
  - LN: x staged fp32->bf16 and x^2 + the a-multiply on the (otherwise
    idle) Pool engine, b2-add on DVE; chunk 0 uses ACT/DVE instead
    (fast path) because its LN is on the critical path.
  - Pipeline: per chunk, S runs two pairs ahead of AV; the previous
    chunk's out-projection interleaves with AV; the next chunk's x DMAs
    issue at iteration top and its LN overlaps the attention phase; the
    drain projection uses a single DVE stt residual.
"""

import numpy as np
import ml_dtypes

import concourse.bass as bass
import concourse.bacc as bacc
import concourse.tile as tile
import concourse.mybir as mybir
from concourse.bass_utils import run_bass_kernel_spmd

F32 = mybir.dt.float32
F32R = mybir.dt.float32r
BF16 = mybir.dt.bfloat16
FP8 = mybir.dt.float8e4
NPBF16 = ml_dtypes.bfloat16
NPFP8 = ml_dtypes.float8_e4m3fn
AF = mybir.ActivationFunctionType
ALU = mybir.AluOpType
DR = mybir.MatmulPerfMode.DoubleRow

DIM = 1024
T = 16384
NCORES = 8
TLOC = T // NCORES          # 2048
CHUNK = 512
NCHUNK = TLOC // CHUNK      # 4
HEADS = 16
DH = 64
WIN = 32
NPAIR = HEADS // 2          # 8 head pairs <-> 128-row tiles
NGRP = CHUNK // 128         # 4 groups of 128 tokens (4 windows each)
KT = DIM // 128             # 8 k-tiles of the contraction dim
NPR = KT // 2               # 4 DoubleRow k-tile pairs
EPS = 1e-5
SCALE = DH ** -0.5          # 0.125
SX = 8.0                    # activation fp8 scale
SW = 64.0                   # weight fp8 scale
SO = 8.0                    # attention-output fp8 scale
RQK = 1.0 / (SX * SW)       # qkv psum scale (folded into exp/ao scales)
RPJ = 1.0 / (SO * SW)       # proj eviction rescale
BIGRAW = 24.0 / (SCALE * RQK * RQK)   # diag shift in raw-S units -> +24

_CACHE = {}


def _build(beta_nonzero: bool, opts: dict | None = None):
    O = dict(wcomp=True, xt_bufs=2, normed_bufs=2, psmm_bufs=2,
             qe_bufs=2, vtok_bufs=1, ao_bufs=2, p_bufs=3, s_bufs=4,
             pt_bufs=6)
    if opts:
        O.update(opts)
    nc = bacc.Bacc("TRN2", target_bir_lowering=False, debug=False,
                   num_devices=NCORES)

    x_d = nc.dram_tensor("x", [DIM, TLOC], F32, kind="ExternalInput").ap()
    wqh_d = nc.dram_tensor("wqkvT8h", [DIM, 3 * DIM], FP8,
                           kind="ExternalInput").ap()
    wql_d = nc.dram_tensor("wqkvT8l", [DIM, 3 * DIM], FP8,
                           kind="ExternalInput").ap()
    woh_d = nc.dram_tensor("woutT8h", [DIM, DIM], FP8,
                           kind="ExternalInput").ap()
    wol_d = nc.dram_tensor("woutT8l", [DIM, DIM], FP8,
                           kind="ExternalInput").ap()
    cos_d = nc.dram_tensor("cosT", [128, CHUNK], BF16,
                           kind="ExternalInput").ap()
    sin_d = nc.dram_tensor("sinT", [128, CHUNK], BF16,
                           kind="ExternalInput").ap()
    ohq_d = nc.dram_tensor("ohq", [4, 128], BF16, kind="ExternalInput").ap()
    ohk_d = nc.dram_tensor("ohk", [4, 128], BF16, kind="ExternalInput").ap()
    onesA_d = nc.dram_tensor("onesA", [128, 33], BF16,
                             kind="ExternalInput").ap()
    onesB_d = nc.dram_tensor("onesB", [128, 33], BF16,
                             kind="ExternalInput").ap()
    oner_d = nc.dram_tensor("onesrow", [1, 128], BF16, kind="ExternalInput").ap()
    qb_d = nc.dram_tensor("qkvbias", [3 * DIM], F32, kind="ExternalInput").ap()
    vb_d = nc.dram_tensor("vbias", [128, DIM], BF16, kind="ExternalInput").ap()
    out_d = nc.dram_tensor("out", [DIM, TLOC], F32, kind="ExternalOutput").ap()

    x_v = x_d.rearrange("(t p) n -> p t n", p=128)       # (128, 8, 2048)
    wqh_v = wqh_d.rearrange("(t p) n -> p t n", p=128)   # (128, 8, 3072)
    wql_v = wql_d.rearrange("(t p) n -> p t n", p=128)
    woh_v = woh_d.rearrange("(t p) n -> p t n", p=128)   # (128, 8, 1024)
    wol_v = wol_d.rearrange("(t p) n -> p t n", p=128)
    qb_v = qb_d.rearrange("(t p) -> p t", p=128)         # (128, 24)
    out_v = out_d.rearrange("(t p) n -> p t n", p=128)   # (128, 8, 2048)

    from contextlib import ExitStack

    with tile.TileContext(nc) as tc:
        with ExitStack() as stk:
            ec = stk.enter_context
            wpool = ec(tc.tile_pool(name="weights", bufs=1))
            cpool = ec(tc.tile_pool(name="consts", bufs=1))
            xtpool = ec(tc.tile_pool(name="xt", bufs=O["xt_bufs"]))
            xfpool = ec(tc.tile_pool(name="xf", bufs=2))
            xsqpool = ec(tc.tile_pool(name="xsq", bufs=1))
            lnrow = ec(tc.tile_pool(name="lnrow", bufs=1))
            lntmp = ec(tc.tile_pool(name="lntmp", bufs=1))
            npool = ec(tc.tile_pool(name="normed", bufs=O["normed_bufs"]))
            qepool = ec(tc.tile_pool(name="qkevict", bufs=2))
            qallpool = ec(tc.tile_pool(name="qall", bufs=O["qe_bufs"]))
            qppool = ec(tc.tile_pool(name="qperm", bufs=2))
            vpool = ec(tc.tile_pool(name="vtok", bufs=O["vtok_bufs"]))
            ppool = ec(tc.tile_pool(name="attnP", bufs=O["p_bufs"]))
            zpool = ec(tc.tile_pool(name="attnZ", bufs=2))
            aopool = ec(tc.tile_pool(name="ao", bufs=O["ao_bufs"]))
            opool = ec(tc.tile_pool(name="outs", bufs=2))
            xrpool = ec(tc.tile_pool(name="xres", bufs=2))
            ps_mm = ec(tc.tile_pool(name="ps_mm", bufs=O["psmm_bufs"],
                                    space="PSUM"))
            ps_ln = ec(tc.tile_pool(name="ps_ln", bufs=1, space="PSUM"))
            ps_s = ec(tc.tile_pool(name="ps_s", bufs=O["s_bufs"], space="PSUM"))
            ps_av = ec(tc.tile_pool(name="ps_av", bufs=1, space="PSUM"))
            ptpool = ec(tc.tile_pool(name="ptp", bufs=O["pt_bufs"]))

            # ---- constants ----
            cos_sb = cpool.tile([128, CHUNK], BF16, tag="cos")
            nc.sync.dma_start(cos_sb, cos_d)
            sin_sb = cpool.tile([128, CHUNK], BF16, tag="sin")
            nc.sync.dma_start(sin_sb, sin_d)
            ohq_sb = cpool.tile([4, 128], BF16, tag="ohq")
            nc.sync.dma_start(ohq_sb, ohq_d)
            ohk_sb = cpool.tile([4, 128], BF16, tag="ohk")
            nc.sync.dma_start(ohk_sb, ohk_d)
            scr_sb = cpool.tile([128, 128], BF16, tag="zscratch")
            onesA_sb = cpool.tile([128, 33], BF16, tag="onesA")
            nc.sync.dma_start(onesA_sb, onesA_d)
            onesB_sb = cpool.tile([128, 33], BF16, tag="onesB")
            nc.sync.dma_start(onesB_sb, onesB_d)
            oner_sb = cpool.tile([1, 128], BF16, tag="onesrow")
            nc.sync.dma_start(oner_sb, oner_d)
            qb_sb = cpool.tile([128, 24], F32, tag="qbias")
            nc.sync.dma_start(qb_sb, qb_v)
            vb_sb = None
            if beta_nonzero:
                vb_sb = cpool.tile([128, DIM], BF16, tag="vbias")
                nc.sync.dma_start(vb_sb, vb_d)
            eps_sb = cpool.tile([1, 1], F32, tag="eps")
            nc.vector.memset(eps_sb, EPS / (SX * SX))
            wqh_sb = wpool.tile([128, KT, 3 * DIM], FP8, tag="wqh")
            wql_sb = None
            if O["wcomp"]:
                wql_sb = wpool.tile([128, KT, 3 * DIM], FP8, tag="wql")
            woh_sb = wpool.tile([128, KT, DIM], FP8, tag="woh")
            wol_sb = None
            if O["wcomp"]:
                wol_sb = wpool.tile([128, KT, DIM], FP8, tag="wol")

            def load_weights():
                # q/k column halves first (matches first-chunk consumption
                # order), then v columns, then the out-proj weights
                for half in range(2):
                    cs = slice(half * DIM, (half + 1) * DIM)
                    nc.sync.dma_start(wqh_sb[:, :, cs], wqh_v[:, :, cs])
                    if wql_sb is not None:
                        nc.sync.dma_start(wql_sb[:, :, cs], wql_v[:, :, cs])
                cs = slice(2 * DIM, 3 * DIM)
                nc.sync.dma_start(wqh_sb[:, :, cs], wqh_v[:, :, cs])
                if wql_sb is not None:
                    nc.sync.dma_start(wql_sb[:, :, cs], wql_v[:, :, cs])
                nc.sync.dma_start(woh_sb, woh_v)
                if wol_sb is not None:
                    nc.sync.dma_start(wol_sb, wol_v)

            def ln_dma(ic, fast=False):
                """DMA x for chunk ic and stage to bf16 (ACT when `fast`:
                chunk 0's LN is on the critical path and Pool serializes)."""
                csl = slice(ic * CHUNK, (ic + 1) * CHUNK)
                xts = xtpool.tile([128, KT, CHUNK], BF16, tag="xt",
                                  name=f"xt{ic}")
                for t in range(KT):
                    xf = xfpool.tile([128, CHUNK], F32, tag="xf")
                    nc.sync.dma_start(xf, x_v[:, t, csl])
                    if fast:
                        nc.scalar.copy(xts[:, t, :], xf)
                    else:
                        nc.gpsimd.tensor_copy(xts[:, t, :], xf)
                return xts

            def ln_stats(xts, fast=False):
                """LN stats + broadcasts -> (a_sb, b2_sb).

                a_sb broadcasts SX/sigma; b2_sb = -mu*SX/sigma, so the
                normalized activations come out pre-scaled by SX for fp8.
                """
                stats = ps_ln.tile([33, CHUNK], F32, tag="lnps")
                for t in range(KT):
                    nc.tensor.matmul(stats, onesA_sb, xts[:, t, :],
                                     start=(t == 0), stop=False)
                for t in range(KT):
                    xsq = xsqpool.tile([128, CHUNK], BF16, tag="xsq")
                    if fast:
                        nc.vector.tensor_mul(xsq, xts[:, t, :], xts[:, t, :])
                    else:
                        nc.gpsimd.tensor_tensor(xsq, xts[:, t, :],
                                                xts[:, t, :], ALU.mult)
                    nc.tensor.matmul(stats, onesB_sb, xsq,
                                     start=False, stop=(t == KT - 1))

                mu = lnrow.tile([1, CHUNK], F32, tag="mu")
                nc.vector.tensor_scalar_mul(mu, stats[0:1, :], 1.0 / DIM)
                var = lnrow.tile([1, CHUNK], F32, tag="var")
                nc.vector.tensor_mul(var, mu, mu)
                nc.vector.scalar_tensor_tensor(var, stats[32:33, :],
                                               1.0 / DIM, var,
                                               ALU.mult, ALU.subtract)
                # sqrt((var+eps)/SX^2) = sigma/SX ; reciprocal -> SX/sigma
                nc.scalar.activation(var, var, AF.Sqrt, bias=eps_sb,
                                     scale=1.0 / (SX * SX))
                a_row = lnrow.tile([1, CHUNK], F32, tag="arow")
                nc.vector.reciprocal(a_row, var)
                b2_row = lnrow.tile([1, CHUNK], F32, tag="b2row")
                nc.vector.scalar_tensor_tensor(b2_row, mu, -1.0, a_row,
                                               ALU.mult, ALU.mult)

                def bcast(row, tag):
                    hi = lnrow.tile([1, CHUNK], BF16, tag=tag + "hi")
                    nc.vector.tensor_copy(hi, row)
                    bc = ps_ln.tile([128, CHUNK], F32, tag="lnps")
                    nc.tensor.matmul(bc, oner_sb, hi, start=True, stop=True)
                    sb = lntmp.tile([128, CHUNK], BF16, tag=tag + "sb",
                                    bufs=1)
                    nc.scalar.copy(sb, bc)
                    return sb

                a_sb = bcast(a_row, "abc")
                b2_sb = bcast(b2_row, "b2bc")
                return a_sb, b2_sb

            def ln_apply(st, fast=False):
                xts = st["xt"]
                a_sb, b2_sb = st["ln"]
                for t in range(KT):
                    tmp = lntmp.tile([128, CHUNK], BF16, tag="lntmp")
                    if fast:
                        nc.vector.tensor_mul(tmp, xts[:, t, :], a_sb)
                    else:
                        nc.gpsimd.tensor_tensor(tmp, xts[:, t, :], a_sb,
                                                ALU.mult)
                    nc.vector.tensor_add(st["normed"][:, t, :], tmp, b2_sb)

            wq_list = [wqh_sb] + ([wql_sb] if O["wcomp"] else [])
            wo_list = [woh_sb] + ([wol_sb] if O["wcomp"] else [])

            def qk_tile(st, jp):
                """fp8 DR projection of q/k tile jp; evict raw to qe."""
                normed = st["normed"]
                ps = ps_mm.tile([128, 2, CHUNK // 2], F32, tag="mm")
                cols = slice(jp * 128, (jp + 1) * 128)
                total = len(wq_list) * NPR
                for h in range(2):
                    hsl = slice(h * 256, (h + 1) * 256)
                    n = 0
                    for wsb in wq_list:
                        for t in range(NPR):
                            nc.tensor.matmul(
                                ps[:, h, :], wsb[:, 2 * t:2 * t + 2, cols],
                                normed[:, 2 * t:2 * t + 2, hsl],
                                start=(n == 0), stop=(n == total - 1),
                                perf_mode=DR)
                            n += 1
                nc.scalar.activation(st["qe"][:, jp, :],
                                     ps.rearrange("p a b -> p (a b)"),
                                     AF.Identity,
                                     bias=qb_sb[:, jp:jp + 1])

            def perm_quarter(st, hs):
                qp = qppool.tile([128, 4, CHUNK], BF16, tag="qp")
                for a in range(4):
                    src = (a // 2) * 64 + ((a % 2) ^ 1) * 32
                    nc.sync.dma_start(
                        qp[a * 32:(a + 1) * 32, :, :],
                        st["qe"][src:src + 32, hs, :])
                return qp

            def rotary_quad(st, hs, qp):
                """rotary for a 4-tile quarter in three batched 2x DVE ops."""
                nq = hs.stop - hs.start
                cosb = cos_sb[:, None, :].to_broadcast((128, nq, CHUNK))
                sinb = sin_sb[:, None, :].to_broadcast((128, nq, CHUNK))
                t1 = qepool.tile([128, 4, CHUNK], BF16, tag="rt1")
                nc.vector.tensor_mul(t1[:, :nq, :], st["qe"][:, hs, :], cosb)
                t2 = qepool.tile([128, 4, CHUNK], BF16, tag="rt2")
                nc.vector.tensor_mul(t2[:, :nq, :], qp[:, :nq, :], sinb)
                nc.vector.tensor_add(st["qe"][:, hs, :], t1[:, :nq, :],
                                     t2[:, :nq, :])

            def v_tile_impl(st, g):
                """token-major V: lhsT = normed (tokens as stationary)."""
                normed = st["normed"]
                gs = slice(g * 128, (g + 1) * 128)
                for hf in range(2):
                    ps = ps_mm.tile([128, 2, CHUNK // 2], F32, tag="mm")
                    for h in range(2):
                        c0 = 2 * DIM + hf * 512 + h * 256
                        n = 0
                        wlist = ([wqh_sb, wql_sb] if O["wcomp"]
                                 else [wqh_sb])
                        total = len(wlist) * NPR
                        for wsb in wlist:
                            for t in range(NPR):
                                nc.tensor.matmul(
                                    ps[:, h, :],
                                    normed[:, 2 * t:2 * t + 2, gs],
                                    wsb[:, 2 * t:2 * t + 2, c0:c0 + 256],
                                    start=(n == 0), stop=(n == total - 1),
                                    perf_mode=DR)
                                n += 1
                    vdst = st["vt"][:, g, hf * 512:(hf + 1) * 512]
                    nc.scalar.activation(vdst,
                                         ps.rearrange("p a b -> p (a b)"),
                                         AF.Identity)
                    if beta_nonzero:
                        nc.vector.scalar_tensor_tensor(
                            vdst, vb_sb[:, hf * 512:(hf + 1) * 512],
                            1.0, vdst, ALU.mult, ALU.add)

            def attn_s(st, p):
                """S matmuls for head pair p, plus the +BIGRAW diag shift
                (window-onehot rank-4 matmul) that makes the off-window
                garbage vanish after exp without an explicit mask."""
                s_ab = []
                for h2 in range(2):
                    s_ps = ps_s.tile([128, CHUNK], F32, tag="s")
                    rs = slice(h2 * 64, (h2 + 1) * 64)
                    for g in range(NGRP):
                        gs = slice(g * 128, (g + 1) * 128)
                        nc.tensor.matmul(
                            s_ps[:, gs], ohq_sb, ohk_sb,
                            start=True, stop=False)
                        nc.tensor.matmul(
                            s_ps[:, gs], st["qe"][rs, p, gs],
                            st["qe"][rs, NPAIR + p, gs],
                            start=False, stop=True)
                    s_ab.append(s_ps)
                st["s"][p] = s_ab

            def attn_soft(st, p):
                """softmax for pair p: exp (scale folds the 512^2 psum
                scale), per-group Z via 4x tensor_scalar accum, reciprocal,
                per-group 4x normalize, block transpose."""
                pts = []
                for h2 in range(2):
                    pe_ = ppool.tile([128, CHUNK], BF16, tag="pexp")
                    nc.scalar.activation(pe_, st["s"][p][h2], AF.Exp,
                                         scale=SCALE * RQK * RQK)
                    z = zpool.tile([128, NGRP], F32, tag="z")
                    for g in range(NGRP):
                        gs = slice(g * 128, (g + 1) * 128)
                        nc.vector.tensor_scalar(
                            scr_sb, pe_[:, gs], 1.0, None, ALU.mult,
                            ALU.add, accum_out=z[:, g:g + 1])
                    rz = zpool.tile([128, NGRP], F32, tag="rz")
                    nc.vector.reciprocal(rz, z)
                    pn = ppool.tile([128, CHUNK], BF16, tag="pn")
                    for g in range(NGRP):
                        gs = slice(g * 128, (g + 1) * 128)
                        nc.vector.tensor_scalar(
                            pn[:, gs], pe_[:, gs], rz[:, g:g + 1], None,
                            ALU.mult)
                    pt = ptpool.tile([128, CHUNK], BF16, tag="pt")
                    nc.vector.transpose(pt, pn)
                    pts.append(pt)
                st["pt"][p] = pts
                st["s"][p] = None

            def attn_av(st, p):
                av = ps_av.tile([128, CHUNK], F32, tag="av")
                for h2 in range(2):
                    cv = slice((2 * p + h2) * DH, (2 * p + h2 + 1) * DH)
                    for g in range(NGRP):
                        gs = slice(g * 128, (g + 1) * 128)
                        nc.tensor.matmul(
                            av[h2 * 64:(h2 + 1) * 64, gs],
                            st["vt"][:, g, cv], st["pt"][p][h2][:, gs],
                            start=True, stop=True,
                            tile_position=(0, h2 * 64))
                nc.scalar.activation(st["ao"][:, p, :], av, AF.Identity,
                                     scale=SO * RQK)
                st["pt"][p] = None

            def proj_tile(st, j, drain=False):
                ic = st["ic"]
                csl = slice(ic * CHUNK, (ic + 1) * CHUNK)
                ps = ps_mm.tile([128, 2, CHUNK // 2], F32, tag="mm")
                ao = st["ao"]
                cols = slice(j * 128, (j + 1) * 128)
                for h in range(2):
                    hsl = slice(h * 256, (h + 1) * 256)
                    n = 0
                    wlist = ([woh_sb, wol_sb] if O["wcomp"] else [woh_sb])
                    total = len(wlist) * NPR
                    for wsb in wlist:
                        for t in range(NPR):
                            nc.tensor.matmul(
                                ps[:, h, :], wsb[:, 2 * t:2 * t + 2, cols],
                                ao[:, 2 * t:2 * t + 2, hsl],
                                start=(n == 0), stop=(n == total - 1),
                                perf_mode=DR)
                            n += 1
                xr = xrpool.tile([128, CHUNK], F32, tag="xr")
                nc.sync.dma_start(xr, x_v[:, j, csl])
                o = opool.tile([128, CHUNK], F32, tag="o")
                if drain:
                    # single-op residual on the (idle at drain) DVE
                    nc.vector.scalar_tensor_tensor(
                        o, ps.rearrange("p a b -> p (a b)"), RPJ, xr,
                        ALU.mult, ALU.add)
                else:
                    nc.scalar.activation(o, ps.rearrange("p a b -> p (a b)"),
                                         AF.Identity, scale=RPJ)
                    nc.gpsimd.tensor_add(o, o, xr)
                nc.sync.dma_start(out_v[:, j, csl], o)

            def new_state(ic, fast=False):
                return {
                    "ic": ic,
                    "xt": ln_dma(ic, fast=fast),
                    "ln": None,
                    "normed": npool.tile([128, KT, CHUNK], FP8,
                                         tag="normed", name=f"normed{ic}"),
                    "qe": qallpool.tile([128, 2 * NPAIR, CHUNK], BF16,
                                        tag="qeall", name=f"qeall{ic}"),
                    "vt": vpool.tile([128, NGRP, DIM], BF16, tag="vtok",
                                     name=f"vtok{ic}"),
                    "ao": aopool.tile([128, NPAIR, CHUNK], FP8, tag="ao",
                                      name=f"ao{ic}"),
                    "s": [None] * NPAIR,
                    "pt": [None] * NPAIR,
                }

            # ---- software pipeline over chunks ----
            # iter ic: qk+rotary for cur; then LN stats/apply for ic+1
            # (x DMAs for ic+1 were queued at the top); then v+attention
            # for cur interleaved, then AV for cur + out-proj for ic-1.
            prev = None
            cur = new_state(0, fast=True)
            cur["ln"] = ln_stats(cur["xt"], fast=True)
            ln_apply(cur, fast=True)
            load_weights()
            for ic in range(NCHUNK):
                nxt = None
                if ic + 1 < NCHUNK:
                    nxt = new_state(ic + 1)   # x DMAs + bf16 staging
                for p in range(NPAIR):
                    qk_tile(cur, p)           # q tile p
                    qk_tile(cur, NPAIR + p)   # k tile p
                    if p % 4 == 3:
                        q0 = p - 3
                        qpa = perm_quarter(cur, slice(q0, q0 + 4))
                        qpb = perm_quarter(cur, slice(NPAIR + q0,
                                                      NPAIR + q0 + 4))
                        rotary_quad(cur, slice(q0, q0 + 4), qpa)
                        rotary_quad(cur, slice(NPAIR + q0, NPAIR + q0 + 4),
                                    qpb)
                if nxt is not None:
                    nxt["ln"] = ln_stats(nxt["xt"])
                    ln_apply(nxt)
                for g in range(NGRP):
                    v_tile_impl(cur, g)
                # S(p) runs two pairs ahead of AV(p) so the softmax chain
                # (ACT exp -> DVE z/norm/transpose) hides behind PE work;
                # the out-proj of the previous chunk fills remaining PE gaps.
                for p in range(NPAIR + 2):
                    if p < NPAIR:
                        attn_s(cur, p)
                        attn_soft(cur, p)
                    if p >= 2:
                        attn_av(cur, p - 2)
                        if prev is not None:
                            proj_tile(prev, p - 2)
                prev = cur
                cur = nxt

            for j in range(KT):
                proj_tile(prev, j, drain=True)

    nc.compile()
    return nc


def _host_constants(w_qkv, w_out, gamma, beta, wcomp=True):
    wg = (w_qkv.astype(np.float32) * gamma.astype(np.float32)[None, :])
    wqT = np.ascontiguousarray(wg.T) * SW                     # (1024, 3072)
    wqh = wqT.astype(NPFP8)
    wql = (wqT - wqh.astype(np.float32)).astype(NPFP8)
    woT = np.ascontiguousarray(w_out.astype(np.float32).T) * SW
    woh = woT.astype(NPFP8)
    wol = (woT - woh.astype(np.float32)).astype(NPFP8)

    # biases live in the 512x-scaled psum/qe domain (scale folding)
    qkvbias = (SX * SW) * (w_qkv.astype(np.float32) @ beta.astype(np.float32)
                           ).astype(np.float32)               # (3072,)
    vbias = np.ascontiguousarray(
        np.broadcast_to(qkvbias[2 * DIM:].astype(NPBF16), (128, DIM)))

    inv_freq = (1.0 / (10000.0 ** (np.arange(0, DH, 2, dtype=np.float64)
                                   / DH))).astype(np.float64)    # (32,)
    p = np.arange(128)
    j = np.arange(CHUNK)
    pos = (j % WIN).astype(np.float64)
    freq = inv_freq[(p % DH) % 32]                               # (128,)
    ang = freq[:, None] * pos[None, :]                           # (128, 512)
    cosT = np.cos(ang).astype(NPBF16)
    sgn = np.where((p % DH) < 32, -1.0, 1.0)
    sinT = (sgn[:, None] * np.sin(ang)).astype(NPBF16)

    oh = (np.arange(128)[None, :] // WIN == np.arange(4)[:, None]
          ).astype(NPBF16)                                       # (4,128)
    ohq = np.ascontiguousarray(oh)
    ohk = np.ascontiguousarray(oh.astype(np.float32) * BIGRAW).astype(NPBF16)

    onesA = np.zeros((128, 33), NPBF16)
    onesA[:, 0] = 1.0
    onesB = np.zeros((128, 33), NPBF16)
    onesB[:, 32] = 1.0
    onesrow = np.ones((1, 128), NPBF16)
    return dict(wqkvT8h=wqh, wqkvT8l=wql, woutT8h=woh, woutT8l=wol,
                qkvbias=qkvbias, vbias=vbias, cosT=cosT, sinT=sinT,
                ohq=ohq, ohk=ohk, onesA=onesA, onesB=onesB, onesrow=onesrow)


def _run(inputs, trace=False, trace_cores=None):
    x = np.asarray(inputs["x"], dtype=np.float32)
    consts = _host_constants(np.asarray(inputs["w_qkv"], np.float32),
                             np.asarray(inputs["w_out"], np.float32),
                             np.asarray(inputs["gamma"], np.float32),
                             np.asarray(inputs["beta"], np.float32))
    beta_nonzero = bool(np.any(np.asarray(inputs["beta"]) != 0))
    key = ("nc", beta_nonzero)
    if key not in _CACHE:
        _CACHE[key] = _build(beta_nonzero)
    nc = _CACHE[key]

    in_maps = []
    for c in range(NCORES):
        m = dict(consts)
        m["x"] = np.ascontiguousarray(x[:, c * TLOC:(c + 1) * TLOC])
        if not beta_nonzero:
            m["vbias"] = np.zeros((128, DIM), NPBF16)
        in_maps.append(m)

    res = run_bass_kernel_spmd(nc, in_maps, list(range(NCORES)),
                               trace=trace,
                               trace_cores=trace_cores)
    out = np.concatenate([res.results[c]["out"] for c in range(NCORES)],
                         axis=1)
    return out, res


def kernel(**inputs):
    out, _ = _run(inputs)
    return out


# revision 89
# speedup vs baseline: 1.0004x; 1.0004x over previous
"""Trainium2 Bass kernel for local windowed MHA (nn_LocalMHA), v2.

Computation (see reference): x (C=1024, T=16384) -> LayerNorm over C ->
QKV proj -> rotary (window-relative) -> per-head attention within windows
of 32 tokens -> out proj -> +x residual.

Sharding: T split across 8 cores (2048 tokens each); windows are local so
no communication is needed. Weights replicated.

v2 changes vs baseline (382us -> 301us in the TimelineSim cost model):
  - QKV and out-projection run in fp8e4 DoubleRow perf mode (2 k-tiles
    per matmul at 0.5 cycles/row, 4x cheaper than bf16) with one-term
    error-feedback weight compensation: W ~= W_hi8 + W_lo8 accumulated
    in the same PSUM group, which drops the weight-side quantization
    error into e4m3 subnormals (~0.1% rms).  Activations are scaled x8
    and weights x64 to stay in e4m3 normal range; the 512x psum scale
    is folded into the exp scale / ao eviction / residual stt, so the
    q/k/v evictions are plain copies.
  - Window masking is free: a rank-4 "onehot" matmul adds +BIGRAW to
    the in-window diagonal of every S block before exp, so off-window
    garbage underflows to 0 after the 1/Z normalize (no mask op).
  - Softmax: exp on ACT; per-group Z via 4x tensor_scalar with
    accum_out into a scratch tile; reciprocal; per-group 4x
    tensor_scalar normalize with the per-partition 1/Z; DVE 32x32
    stream transpose (exact for the block-diagonal P).
  - LN: x staged fp32->bf16 and x^2 + the a-multiply on the (otherwise
    idle) Pool engine, b2-add on DVE; chunk 0 uses ACT/DVE instead
    (fast path) because its LN is on the critical path.
  - Pipeline: per chunk, S runs two pairs ahead of AV; the previous
    chunk's out-projection interleaves with AV; the next chunk's x DMAs
    issue at iteration top and its LN overlaps the attention phase; the
    drain projection uses a single DVE stt residual.
"""

import numpy as np
import ml_dtypes

import concourse.bass as bass
import concourse.bacc as bacc
import concourse.tile as tile
import concourse.mybir as mybir
from concourse.bass_utils import run_bass_kernel_spmd

F32 = mybir.dt.float32
F32R = mybir.dt.float32r
BF16 = mybir.dt.bfloat16
FP8 = mybir.dt.float8e4
NPBF16 = ml_dtypes.bfloat16
NPFP8 = ml_dtypes.float8_e4m3fn
AF = mybir.ActivationFunctionType
ALU = mybir.AluOpType
DR = mybir.MatmulPerfMode.DoubleRow

DIM = 1024
T = 16384
NCORES = 8
TLOC = T // NCORES          # 2048
CHUNK = 512
NCHUNK = TLOC // CHUNK      # 4
HEADS = 16
DH = 64
WIN = 32
NPAIR = HEADS // 2          # 8 head pairs <-> 128-row tiles
NGRP = CHUNK // 128         # 4 groups of 128 tokens (4 windows each)
KT = DIM // 128             # 8 k-tiles of the contraction dim
NPR = KT // 2               # 4 DoubleRow k-tile pairs
EPS = 1e-5
SCALE = DH ** -0.5          # 0.125
SX = 8.0                    # activation fp8 scale
SW = 64.0                   # weight fp8 scale
SO = 8.0                    # attention-output fp8 scale
RQK = 1.0 / (SX * SW)       # qkv psum scale (folded into exp/ao scales)
RPJ = 1.0 / (SO * SW)       # proj eviction rescale
BIGRAW = 24.0 / (SCALE * RQK * RQK)   # diag shift in raw-S units -> +24

_CACHE = {}


def _build(beta_nonzero: bool, opts: dict | None = None):
    O = dict(wcomp=True, xt_bufs=2, normed_bufs=2, psmm_bufs=2,
             qe_bufs=2, vtok_bufs=1, ao_bufs=2, p_bufs=3, s_bufs=4,
             pt_bufs=6)
    if opts:
        O.update(opts)
    nc = bacc.Bacc("TRN2", target_bir_lowering=False, debug=False,
                   num_devices=NCORES)

    x_d = nc.dram_tensor("x", [DIM, TLOC], F32, kind="ExternalInput").ap()
    wqh_d = nc.dram_tensor("wqkvT8h", [DIM, 3 * DIM], FP8,
                           kind="ExternalInput").ap()
    wql_d = nc.dram_tensor("wqkvT8l", [DIM, 3 * DIM], FP8,
                           kind="ExternalInput").ap()
    woh_d = nc.dram_tensor("woutT8h", [DIM, DIM], FP8,
                           kind="ExternalInput").ap()
    wol_d = nc.dram_tensor("woutT8l", [DIM, DIM], FP8,
                           kind="ExternalInput").ap()
    cos_d = nc.dram_tensor("cosT", [128, CHUNK], BF16,
                           kind="ExternalInput").ap()
    sin_d = nc.dram_tensor("sinT", [128, CHUNK], BF16,
                           kind="ExternalInput").ap()
    ohq_d = nc.dram_tensor("ohq", [4, 128], BF16, kind="ExternalInput").ap()
    ohk_d = nc.dram_tensor("ohk", [4, CHUNK], BF16,
                           kind="ExternalInput").ap()
    onesA_d = nc.dram_tensor("onesA", [128, 33], BF16,
                             kind="ExternalInput").ap()
    onesB_d = nc.dram_tensor("onesB", [128, 33], BF16,
                             kind="ExternalInput").ap()
    oner_d = nc.dram_tensor("onesrow", [1, 128], BF16, kind="ExternalInput").ap()
    qb_d = nc.dram_tensor("qkvbias", [3 * DIM], F32, kind="ExternalInput").ap()
    vb_d = nc.dram_tensor("vbias", [128, DIM], BF16, kind="ExternalInput").ap()
    out_d = nc.dram_tensor("out", [DIM, TLOC], F32, kind="ExternalOutput").ap()

    x_v = x_d.rearrange("(t p) n -> p t n", p=128)       # (128, 8, 2048)
    wqh_v = wqh_d.rearrange("(t p) n -> p t n", p=128)   # (128, 8, 3072)
    wql_v = wql_d.rearrange("(t p) n -> p t n", p=128)
    woh_v = woh_d.rearrange("(t p) n -> p t n", p=128)   # (128, 8, 1024)
    wol_v = wol_d.rearrange("(t p) n -> p t n", p=128)
    qb_v = qb_d.rearrange("(t p) -> p t", p=128)         # (128, 24)
    out_v = out_d.rearrange("(t p) n -> p t n", p=128)   # (128, 8, 2048)

    from contextlib import ExitStack

    with tile.TileContext(nc) as tc:
        with ExitStack() as stk:
            ec = stk.enter_context
            wpool = ec(tc.tile_pool(name="weights", bufs=1))
            cpool = ec(tc.tile_pool(name="consts", bufs=1))
            xtpool = ec(tc.tile_pool(name="xt", bufs=O["xt_bufs"]))
            xfpool = ec(tc.tile_pool(name="xf", bufs=2))
            xsqpool = ec(tc.tile_pool(name="xsq", bufs=1))
            lnrow = ec(tc.tile_pool(name="lnrow", bufs=1))
            lntmp = ec(tc.tile_pool(name="lntmp", bufs=1))
            npool = ec(tc.tile_pool(name="normed", bufs=O["normed_bufs"]))
            qepool = ec(tc.tile_pool(name="qkevict", bufs=2))
            qallpool = ec(tc.tile_pool(name="qall", bufs=O["qe_bufs"]))
            qppool = ec(tc.tile_pool(name="qperm", bufs=2))
            vpool = ec(tc.tile_pool(name="vtok", bufs=O["vtok_bufs"]))
            ppool = ec(tc.tile_pool(name="attnP", bufs=O["p_bufs"]))
            zpool = ec(tc.tile_pool(name="attnZ", bufs=2))
            aopool = ec(tc.tile_pool(name="ao", bufs=O["ao_bufs"]))
            opool = ec(tc.tile_pool(name="outs", bufs=2))
            xrpool = ec(tc.tile_pool(name="xres", bufs=2))
            ps_mm = ec(tc.tile_pool(name="ps_mm", bufs=O["psmm_bufs"],
                                    space="PSUM"))
            ps_ln = ec(tc.tile_pool(name="ps_ln", bufs=1, space="PSUM"))
            ps_s = ec(tc.tile_pool(name="ps_s", bufs=O["s_bufs"], space="PSUM"))
            ps_av = ec(tc.tile_pool(name="ps_av", bufs=1, space="PSUM"))
            ptpool = ec(tc.tile_pool(name="ptp", bufs=O["pt_bufs"]))

            # ---- constants ----
            cos_sb = cpool.tile([128, CHUNK], BF16, tag="cos")
            nc.sync.dma_start(cos_sb, cos_d)
            sin_sb = cpool.tile([128, CHUNK], BF16, tag="sin")
            nc.sync.dma_start(sin_sb, sin_d)
            ohq_sb = cpool.tile([4, 128], BF16, tag="ohq")
            nc.sync.dma_start(ohq_sb, ohq_d)
            ohk_sb = cpool.tile([4, CHUNK], BF16, tag="ohk")
            nc.sync.dma_start(ohk_sb, ohk_d)
            scr_sb = cpool.tile([128, 128], BF16, tag="zscratch")
            onesA_sb = cpool.tile([128, 33], BF16, tag="onesA")
            nc.sync.dma_start(onesA_sb, onesA_d)
            onesB_sb = cpool.tile([128, 33], BF16, tag="onesB")
            nc.sync.dma_start(onesB_sb, onesB_d)
            oner_sb = cpool.tile([1, 128], BF16, tag="onesrow")
            nc.sync.dma_start(oner_sb, oner_d)
            qb_sb = cpool.tile([128, 24], F32, tag="qbias")
            nc.sync.dma_start(qb_sb, qb_v)
            vb_sb = None
            if beta_nonzero:
                vb_sb = cpool.tile([128, DIM], BF16, tag="vbias")
                nc.sync.dma_start(vb_sb, vb_d)
            eps_sb = cpool.tile([1, 1], F32, tag="eps")
            nc.vector.memset(eps_sb, EPS / (SX * SX))
            wqh_sb = wpool.tile([128, KT, 3 * DIM], FP8, tag="wqh")
            wql_sb = None
            if O["wcomp"]:
                wql_sb = wpool.tile([128, KT, 3 * DIM], FP8, tag="wql")
            woh_sb = wpool.tile([128, KT, DIM], FP8, tag="woh")
            wol_sb = None
            if O["wcomp"]:
                wol_sb = wpool.tile([128, KT, DIM], FP8, tag="wol")

            def load_weights():
                # q/k column halves first (matches first-chunk consumption
                # order), then v columns, then the out-proj weights
                for half in range(2):
                    cs = slice(half * DIM, (half + 1) * DIM)
                    nc.sync.dma_start(wqh_sb[:, :, cs], wqh_v[:, :, cs])
                    if wql_sb is not None:
                        nc.sync.dma_start(wql_sb[:, :, cs], wql_v[:, :, cs])
                cs = slice(2 * DIM, 3 * DIM)
                nc.sync.dma_start(wqh_sb[:, :, cs], wqh_v[:, :, cs])
                if wql_sb is not None:
                    nc.sync.dma_start(wql_sb[:, :, cs], wql_v[:, :, cs])
                nc.sync.dma_start(woh_sb, woh_v)
                if wol_sb is not None:
                    nc.sync.dma_start(wol_sb, wol_v)

            def ln_dma(ic, fast=False):
                """DMA x for chunk ic and stage to bf16 (ACT when `fast`:
                chunk 0's LN is on the critical path and Pool serializes)."""
                csl = slice(ic * CHUNK, (ic + 1) * CHUNK)
                xts = xtpool.tile([128, KT, CHUNK], BF16, tag="xt",
                                  name=f"xt{ic}")
                for t in range(KT):
                    xf = xfpool.tile([128, CHUNK], F32, tag="xf")
                    nc.sync.dma_start(xf, x_v[:, t, csl])
                    if fast:
                        nc.scalar.copy(xts[:, t, :], xf)
                    else:
                        nc.gpsimd.tensor_copy(xts[:, t, :], xf)
                return xts

            def ln_stats(xts, fast=False):
                """LN stats + broadcasts -> (a_sb, b2_sb).

                a_sb broadcasts SX/sigma; b2_sb = -mu*SX/sigma, so the
                normalized activations come out pre-scaled by SX for fp8.
                """
                stats = ps_ln.tile([33, CHUNK], F32, tag="lnps")
                for t in range(KT):
                    nc.tensor.matmul(stats, onesA_sb, xts[:, t, :],
                                     start=(t == 0), stop=False)
                for t in range(KT):
                    xsq = xsqpool.tile([128, CHUNK], BF16, tag="xsq")
                    if fast:
                        nc.vector.tensor_mul(xsq, xts[:, t, :], xts[:, t, :])
                    else:
                        nc.gpsimd.tensor_tensor(xsq, xts[:, t, :],
                                                xts[:, t, :], ALU.mult)
                    nc.tensor.matmul(stats, onesB_sb, xsq,
                                     start=False, stop=(t == KT - 1))

                mu = lnrow.tile([1, CHUNK], F32, tag="mu")
                nc.vector.tensor_scalar_mul(mu, stats[0:1, :], 1.0 / DIM)
                var = lnrow.tile([1, CHUNK], F32, tag="var")
                nc.vector.tensor_mul(var, mu, mu)
                nc.vector.scalar_tensor_tensor(var, stats[32:33, :],
                                               1.0 / DIM, var,
                                               ALU.mult, ALU.subtract)
                # sqrt((var+eps)/SX^2) = sigma/SX ; reciprocal -> SX/sigma
                nc.scalar.activation(var, var, AF.Sqrt, bias=eps_sb,
                                     scale=1.0 / (SX * SX))
                a_row = lnrow.tile([1, CHUNK], F32, tag="arow")
                nc.vector.reciprocal(a_row, var)
                b2_row = lnrow.tile([1, CHUNK], F32, tag="b2row")
                nc.vector.scalar_tensor_tensor(b2_row, mu, -1.0, a_row,
                                               ALU.mult, ALU.mult)

                def bcast(row, tag):
                    hi = lnrow.tile([1, CHUNK], BF16, tag=tag + "hi")
                    nc.vector.tensor_copy(hi, row)
                    bc = ps_ln.tile([128, CHUNK], F32, tag="lnps")
                    nc.tensor.matmul(bc, oner_sb, hi, start=True, stop=True)
                    sb = lntmp.tile([128, CHUNK], BF16, tag=tag + "sb",
                                    bufs=1)
                    nc.scalar.copy(sb, bc)
                    return sb

                a_sb = bcast(a_row, "abc")
                b2_sb = bcast(b2_row, "b2bc")
                return a_sb, b2_sb

            def ln_apply(st, fast=False):
                xts = st["xt"]
                a_sb, b2_sb = st["ln"]
                for t in range(KT):
                    tmp = lntmp.tile([128, CHUNK], BF16, tag="lntmp")
                    if fast:
                        nc.vector.tensor_mul(tmp, xts[:, t, :], a_sb)
                    else:
                        nc.gpsimd.tensor_tensor(tmp, xts[:, t, :], a_sb,
                                                ALU.mult)
                    nc.vector.tensor_add(st["normed"][:, t, :], tmp, b2_sb)

            wq_list = [wqh_sb] + ([wql_sb] if O["wcomp"] else [])
            wo_list = [woh_sb] + ([wol_sb] if O["wcomp"] else [])

            def qk_tile(st, jp):
                """fp8 DR projection of q/k tile jp; evict raw to qe."""
                normed = st["normed"]
                ps = ps_mm.tile([128, 2, CHUNK // 2], F32, tag="mm")
                cols = slice(jp * 128, (jp + 1) * 128)
                total = len(wq_list) * NPR
                for h in range(2):
                    hsl = slice(h * 256, (h + 1) * 256)
                    n = 0
                    for wsb in wq_list:
                        for t in range(NPR):
                            nc.tensor.matmul(
                                ps[:, h, :], wsb[:, 2 * t:2 * t + 2, cols],
                                normed[:, 2 * t:2 * t + 2, hsl],
                                start=(n == 0), stop=(n == total - 1),
                                perf_mode=DR)
                            n += 1
                nc.scalar.activation(st["qe"][:, jp, :],
                                     ps.rearrange("p a b -> p (a b)"),
                                     AF.Identity,
                                     bias=qb_sb[:, jp:jp + 1])

            def perm_quarter(st, hs):
                qp = qppool.tile([128, 4, CHUNK], BF16, tag="qp")
                for a in range(4):
                    src = (a // 2) * 64 + ((a % 2) ^ 1) * 32
                    nc.sync.dma_start(
                        qp[a * 32:(a + 1) * 32, :, :],
                        st["qe"][src:src + 32, hs, :])
                return qp

            def rotary_quad(st, hs, qp):
                """rotary for a 4-tile quarter in three batched 2x DVE ops."""
                nq = hs.stop - hs.start
                cosb = cos_sb[:, None, :].to_broadcast((128, nq, CHUNK))
                sinb = sin_sb[:, None, :].to_broadcast((128, nq, CHUNK))
                t1 = qepool.tile([128, 4, CHUNK], BF16, tag="rt1")
                nc.vector.tensor_mul(t1[:, :nq, :], st["qe"][:, hs, :], cosb)
                t2 = qepool.tile([128, 4, CHUNK], BF16, tag="rt2")
                nc.vector.tensor_mul(t2[:, :nq, :], qp[:, :nq, :], sinb)
                nc.vector.tensor_add(st["qe"][:, hs, :], t1[:, :nq, :],
                                     t2[:, :nq, :])

            def v_tile_impl(st, g):
                """token-major V: lhsT = normed (tokens as stationary)."""
                normed = st["normed"]
                gs = slice(g * 128, (g + 1) * 128)
                for hf in range(2):
                    ps = ps_mm.tile([128, 2, CHUNK // 2], F32, tag="mm")
                    for h in range(2):
                        c0 = 2 * DIM + hf * 512 + h * 256
                        n = 0
                        wlist = ([wqh_sb, wql_sb] if O["wcomp"]
                                 else [wqh_sb])
                        total = len(wlist) * NPR
                        for wsb in wlist:
                            for t in range(NPR):
                                nc.tensor.matmul(
                                    ps[:, h, :],
                                    normed[:, 2 * t:2 * t + 2, gs],
                                    wsb[:, 2 * t:2 * t + 2, c0:c0 + 256],
                                    start=(n == 0), stop=(n == total - 1),
                                    perf_mode=DR)
                                n += 1
                    vdst = st["vt"][:, g, hf * 512:(hf + 1) * 512]
                    nc.scalar.activation(vdst,
                                         ps.rearrange("p a b -> p (a b)"),
                                         AF.Identity)
                    if beta_nonzero:
                        nc.vector.scalar_tensor_tensor(
                            vdst, vb_sb[:, hf * 512:(hf + 1) * 512],
                            1.0, vdst, ALU.mult, ALU.add)

            def attn_s(st, p):
                """S matmuls for head pair p, plus the +BIGRAW diag shift
                (window-onehot rank-4 matmul) that makes the off-window
                garbage vanish after exp without an explicit mask."""
                s_ab = []
                for h2 in range(2):
                    s_ps = ps_s.tile([128, CHUNK], F32, tag="s")
                    rs = slice(h2 * 64, (h2 + 1) * 64)
                    nc.tensor.matmul(s_ps, ohq_sb, ohk_sb,
                                     start=True, stop=False)
                    for g in range(NGRP):
                        gs = slice(g * 128, (g + 1) * 128)
                        nc.tensor.matmul(
                            s_ps[:, gs], st["qe"][rs, p, gs],
                            st["qe"][rs, NPAIR + p, gs],
                            start=False, stop=(g == NGRP - 1))
                    s_ab.append(s_ps)
                st["s"][p] = s_ab

            def attn_soft(st, p):
                """softmax for pair p: exp (scale folds the 512^2 psum
                scale), per-group Z via 4x tensor_scalar accum, reciprocal,
                per-group 4x normalize, block transpose."""
                pts = []
                for h2 in range(2):
                    pe_ = ppool.tile([128, CHUNK], BF16, tag="pexp")
                    nc.scalar.activation(pe_, st["s"][p][h2], AF.Exp,
                                         scale=SCALE * RQK * RQK)
                    z = zpool.tile([128, NGRP], F32, tag="z")
                    for g in range(NGRP):
                        gs = slice(g * 128, (g + 1) * 128)
                        nc.vector.tensor_scalar(
                            scr_sb, pe_[:, gs], 1.0, None, ALU.mult,
                            ALU.add, accum_out=z[:, g:g + 1])
                    rz = zpool.tile([128, NGRP], F32, tag="rz")
                    nc.vector.reciprocal(rz, z)
                    pn = ppool.tile([128, CHUNK], BF16, tag="pn")
                    for g in range(NGRP):
                        gs = slice(g * 128, (g + 1) * 128)
                        nc.vector.tensor_scalar(
                            pn[:, gs], pe_[:, gs], rz[:, g:g + 1], None,
                            ALU.mult)
                    pt = ptpool.tile([128, CHUNK], BF16, tag="pt")
                    nc.vector.transpose(pt, pn)
                    pts.append(pt)
                st["pt"][p] = pts
                st["s"][p] = None

            def attn_av(st, p):
                av = ps_av.tile([128, CHUNK], F32, tag="av")
                for h2 in range(2):
                    cv = slice((2 * p + h2) * DH, (2 * p + h2 + 1) * DH)
                    for g in range(NGRP):
                        gs = slice(g * 128, (g + 1) * 128)
                        nc.tensor.matmul(
                            av[h2 * 64:(h2 + 1) * 64, gs],
                            st["vt"][:, g, cv], st["pt"][p][h2][:, gs],
                            start=True, stop=True,
                            tile_position=(0, h2 * 64))
                nc.scalar.activation(st["ao"][:, p, :], av, AF.Identity,
                                     scale=SO * RQK)
                st["pt"][p] = None

            def proj_tile(st, j, drain=False):
                ic = st["ic"]
                csl = slice(ic * CHUNK, (ic + 1) * CHUNK)
                ps = ps_mm.tile([128, 2, CHUNK // 2], F32, tag="mm")
                ao = st["ao"]
                cols = slice(j * 128, (j + 1) * 128)
                for h in range(2):
                    hsl = slice(h * 256, (h + 1) * 256)
                    n = 0
                    wlist = ([woh_sb, wol_sb] if O["wcomp"] else [woh_sb])
                    total = len(wlist) * NPR
                    for wsb in wlist:
                        for t in range(NPR):
                            nc.tensor.matmul(
                                ps[:, h, :], wsb[:, 2 * t:2 * t + 2, cols],
                                ao[:, 2 * t:2 * t + 2, hsl],
                                start=(n == 0), stop=(n == total - 1),
                                perf_mode=DR)
                            n += 1
                xr = xrpool.tile([128, CHUNK], F32, tag="xr")
                nc.sync.dma_start(xr, x_v[:, j, csl])
                o = opool.tile([128, CHUNK], F32, tag="o")
                if drain:
                    # single-op residual on the (idle at drain) DVE
                    nc.vector.scalar_tensor_tensor(
                        o, ps.rearrange("p a b -> p (a b)"), RPJ, xr,
                        ALU.mult, ALU.add)
                else:
                    nc.scalar.activation(o, ps.rearrange("p a b -> p (a b)"),
                                         AF.Identity, scale=RPJ)
                    nc.gpsimd.tensor_add(o, o, xr)
                nc.sync.dma_start(out_v[:, j, csl], o)

            def new_state(ic, fast=False):
                return {
                    "ic": ic,
                    "xt": ln_dma(ic, fast=fast),
                    "ln": None,
                    "normed": npool.tile([128, KT, CHUNK], FP8,
                                         tag="normed", name=f"normed{ic}"),
                    "qe": qallpool.tile([128, 2 * NPAIR, CHUNK], BF16,
                                        tag="qeall", name=f"qeall{ic}"),
                    "vt": vpool.tile([128, NGRP, DIM], BF16, tag="vtok",
                                     name=f"vtok{ic}"),
                    "ao": aopool.tile([128, NPAIR, CHUNK], FP8, tag="ao",
                                      name=f"ao{ic}"),
                    "s": [None] * NPAIR,
                    "pt": [None] * NPAIR,
                }

            # ---- software pipeline over chunks ----
            # iter ic: qk+rotary for cur; then LN stats/apply for ic+1
            # (x DMAs for ic+1 were queued at the top); then v+attention
            # for cur interleaved, then AV for cur + out-proj for ic-1.
            prev = None
            cur = new_state(0, fast=True)
            cur["ln"] = ln_stats(cur["xt"], fast=True)
            ln_apply(cur, fast=True)
            load_weights()
            for ic in range(NCHUNK):
                nxt = None
                if ic + 1 < NCHUNK:
                    nxt = new_state(ic + 1)   # x DMAs + bf16 staging
                for p in range(NPAIR):
                    qk_tile(cur, p)           # q tile p
                    qk_tile(cur, NPAIR + p)   # k tile p
                    if p % 4 == 3:
                        q0 = p - 3
                        qpa = perm_quarter(cur, slice(q0, q0 + 4))
                        qpb = perm_quarter(cur, slice(NPAIR + q0,
                                                      NPAIR + q0 + 4))
                        rotary_quad(cur, slice(q0, q0 + 4), qpa)
                        rotary_quad(cur, slice(NPAIR + q0, NPAIR + q0 + 4),
                                    qpb)
                if nxt is not None:
                    nxt["ln"] = ln_stats(nxt["xt"])
                    ln_apply(nxt)
                for g in range(NGRP):
                    v_tile_impl(cur, g)
                # S(p) runs two pairs ahead of AV(p) so the softmax chain
                # (ACT exp -> DVE z/norm/transpose) hides behind PE work;
                # the out-proj of the previous chunk fills remaining PE gaps.
                for p in range(NPAIR + 2):
                    if p < NPAIR:
                        attn_s(cur, p)
                        attn_soft(cur, p)
                    if p >= 2:
                        attn_av(cur, p - 2)
                        if prev is not None:
                            proj_tile(prev, p - 2)
                prev = cur
                cur = nxt

            for j in range(KT):
                proj_tile(prev, j, drain=True)

    nc.compile()
    return nc


def _host_constants(w_qkv, w_out, gamma, beta, wcomp=True):
    wg = (w_qkv.astype(np.float32) * gamma.astype(np.float32)[None, :])
    wqT = np.ascontiguousarray(wg.T) * SW                     # (1024, 3072)
    wqh = wqT.astype(NPFP8)
    wql = (wqT - wqh.astype(np.float32)).astype(NPFP8)
    woT = np.ascontiguousarray(w_out.astype(np.float32).T) * SW
    woh = woT.astype(NPFP8)
    wol = (woT - woh.astype(np.float32)).astype(NPFP8)

    # biases live in the 512x-scaled psum/qe domain (scale folding)
    qkvbias = (SX * SW) * (w_qkv.astype(np.float32) @ beta.astype(np.float32)
                           ).astype(np.float32)               # (3072,)
    vbias = np.ascontiguousarray(
        np.broadcast_to(qkvbias[2 * DIM:].astype(NPBF16), (128, DIM)))

    inv_freq = (1.0 / (10000.0 ** (np.arange(0, DH, 2, dtype=np.float64)
                                   / DH))).astype(np.float64)    # (32,)
    p = np.arange(128)
    j = np.arange(CHUNK)
    pos = (j % WIN).astype(np.float64)
    freq = inv_freq[(p % DH) % 32]                               # (128,)
    ang = freq[:, None] * pos[None, :]                           # (128, 512)
    cosT = np.cos(ang).astype(NPBF16)
    sgn = np.where((p % DH) < 32, -1.0, 1.0)
    sinT = (sgn[:, None] * np.sin(ang)).astype(NPBF16)

    oh = (np.arange(128)[None, :] // WIN == np.arange(4)[:, None]
          ).astype(NPBF16)                                       # (4,128)
    ohq = np.ascontiguousarray(oh)
    ohk = np.ascontiguousarray(
        np.tile(oh.astype(np.float32) * BIGRAW, (1, 4))).astype(NPBF16)

    onesA = np.zeros((128, 33), NPBF16)
    onesA[:, 0] = 1.0
    onesB = np.zeros((128, 33), NPBF16)
    onesB[:, 32] = 1.0
    onesrow = np.ones((1, 128), NPBF16)
    return dict(wqkvT8h=wqh, wqkvT8l=wql, woutT8h=woh, woutT8l=wol,
                qkvbias=qkvbias, vbias=vbias, cosT=cosT, sinT=sinT,
                ohq=ohq, ohk=ohk, onesA=onesA, onesB=onesB, onesrow=onesrow)


def _run(inputs, trace=False, trace_cores=None):
    x = np.asarray(inputs["x"], dtype=np.float32)
    consts = _host_constants(np.asarray(inputs["w_qkv"], np.float32),
                             np.asarray(inputs["w_out"], np.float32),
                             np.asarray(inputs["gamma"], np.float32),
                             np.asarray(inputs["beta"], np.float32))
    beta_nonzero = bool(np.any(np.asarray(inputs["beta"]) != 0))
    key = ("nc", beta_nonzero)
    if key not in _CACHE:
        _CACHE[key] = _build(beta_nonzero)
    nc = _CACHE[key]

    in_maps = []
    for c in range(NCORES):
        m = dict(consts)
        m["x"] = np.ascontiguousarray(x[:, c * TLOC:(c + 1) * TLOC])
        if not beta_nonzero:
            m["vbias"] = np.zeros((128, DIM), NPBF16)
        in_maps.append(m)

    res = run_bass_kernel_spmd(nc, in_maps, list(range(NCORES)),
                               trace=trace,
                               trace_cores=trace_cores)
    out = np.concatenate([res.results[c]["out"] for c in range(NCORES)],
                         axis=1)
    return out, res


def kernel(**inputs):
    out, _ = _run(inputs)
    return out
